# revision 24
# baseline (speedup 1.0000x reference)
"""MoE MLP (2 experts, top-1 routing) Trainium2 kernel.

Sparse dispatch: tokens are sorted by routed expert; cores 0-3 process
expert-0 tokens, cores 4-7 expert-1 tokens, each with per-core capacity
T = ceil(max(n0,n1)/4) rounded up to a multiple of 16 (compiled per T,
cached; T<=640 else dense fallback).  Top-1 routing means no cross-core
combine: the host scatters rows back by token index.

Routing-weight folding: s(n) = top-prob of token n.  leaky_relu is
positively homogeneous and squared, so
  s * sq(lrelu(x@W1.T)) @ W2.T == sq(lrelu((sqrt(s)*x)@W1.T)) @ W2.T
and sqrt(s) is folded into x on the host.

Device program (per core, host-pretransposed, bf16 in / fp32 out):
  xsT [D,T], wfcT [D,H], wpjT [H,D] -> yT [D,T]
  hT = wfc @ xs                 (PSUM fp32, contraction D, 128-blocks)
  aT = Square(Lrelu(hT, 0.5))   (2 ACT ops, bf16)
  yT = wpj @ aT                 (contraction H)

Perf notes (from NTFF traces; baseline 657us -> 467us, MFU 89%):
- ~3.5us of dummy matmuls on zeroed tiles run during the DMA lead-in to
  flip the PE HAM clock-gate to 8/8 before real work; first-use data
  (xs k-blocks, first wfc panel quarters) is DMA'd at the finest grain
  across both rings so the first matmul starts ~8us earlier.
- T is capped at 512 so every matmul has a single 512-wide moving tile:
  at N=512 the 116ns LDWEIGHTS hides completely under the 213ns matmul
  stream (measured spacing 216ns/MM).  At N<512 (T>512 -> paired
  t-tiles sharing the stationary operand) ~28ns/MM of LDWEIGHTS stays
  exposed unless _dedup_ldweights removes the repeated loads (walrus's
  own --enable-ldw-opt rejects Bacc's pre-split InstLdweights).
- wfc panels stream on the sync HWDGE ring, wpj panels prefetch on the
  scalar ring, gated behind early phase-1 progress: ungated they all
  hoist to t=0 and crowd out the lead-in-critical xs/wfc loads (+14us
  to the first matmul); panels beyond the pool depth are throttled by
  buffer recycling (gating those deadlocks the ACT FIFO on phase 2).
- xs is loaded in 4 k-chunks alternating across both rings so the first
  matmul only waits for ~1MB, not the full input + first panel.
- Lrelu's alpha immediate is ignored by the ACT table (measured), hence
  the 3-op Relu + add + Square(scale=0.5) chain.
"""

import numpy as np
import ml_dtypes

P = 128
DIM = 2048
HID = 8192
NEXP = 2
NCORES = 8
NTOK = 4096
TPC_DENSE = 1024
T_MAX_SPARSE = 640
T_CAP = 512        # device capacity per core; overflow spills to host
SPILL_MAX = 256    # max host-spilled tokens per expert (~8 sigma)

LDW_DEDUP = True   # drop InstLdweights whose AP repeats the previous load
USE_LRELU = False  # HW ignores the Lrelu alpha immediate (measured rel err
                   # 0.246 == alpha dropped); keep the 3-op Relu/add/Square

_NC_CACHE = {}
_RUN_CACHE = {}


# --------------------------------------------------------------------------
# BIR pass: drop redundant LDWEIGHTS
# --------------------------------------------------------------------------
def _dedup_ldweights(nc):
    """Remove InstLdweights whose weights AP is byte-identical to the
    previous InstLdweights in the same block.  Matmuls between them are
    non-self-loading (ldweights=False) and the PE stationary registers
    persist across matmuls, so the reload is redundant; walrus's own
    dedup pass (--enable-ldw-opt) rejects pre-split InstLdweights, hence
    this bass-level version.  Deps of the removed load are merged into
    the survivor and dangling dep names are remapped."""
    removed = 0
    for f in nc.m.functions:
        for blk in f.blocks:
            il = blk.instructions
            keep, last, remap = [], None, {}
            for i in il:
                tn = type(i).__name__
                if tn == "InstLdweights":
                    if i.is_transpose or i.perf_mode:
                        last = None
                    else:
                        key = str(i.ins[0])
                        if last is not None and key == last[1]:
                            last[0].merge_dependencies_from(i)
                            remap[i.name] = last[0].name
                            removed += 1
                            continue
                        last = (i, key)
                elif tn == "InstMatmult":
                    if i.is_transpose or i.ldweights:
                        last = None
                keep.append(i)
            if remap:
                for i in keep:
                    i.remap_dependency_names(remap)
            il[:] = keep
    return removed


# --------------------------------------------------------------------------
# device program
# --------------------------------------------------------------------------
def _build_nc(D, H, T, tbs):
    import concourse.mybir as mybir
    import concourse.tile as tile
    from concourse import bacc

    dt = mybir.dt
    nc = bacc.Bacc(None, target_bir_lowering=False)
    xsT = nc.dram_tensor("xsT", [D, T], dt.bfloat16, kind="ExternalInput")
    wfcT = nc.dram_tensor("wfcT", [D, H], dt.bfloat16, kind="ExternalInput")
    wpjT = nc.dram_tensor("wpjT", [H, D], dt.bfloat16, kind="ExternalInput")
    yT = nc.dram_tensor("yT", [D, T], dt.float32, kind="ExternalOutput")

    assert sum(tbs) == T and all(tb <= 512 for tb in tbs)
    toff = [sum(tbs[:i]) for i in range(len(tbs))]
    KB1 = D // P          # fc contraction blocks
    KB2 = H // P          # proj contraction blocks (h-blocks)
    XCHUNK = 4            # xs load split (per-kb chunks per ring)

    xsT_r = xsT.rearrange("(kb p) t -> p kb t", p=P)
    wfcT_r = wfcT.rearrange("(kb p) h -> p kb h", p=P)
    wpjT_r = wpjT.rearrange("(kb p) d -> p kb d", p=P)
    yT_r = yT.rearrange("(db p) t -> p db t", p=P)

    with tile.TileContext(nc) as tc:
        with tc.tile_pool(name="a", bufs=1) as a_pool, \
             tc.tile_pool(name="xs", bufs=1) as xs_pool, \
             tc.tile_pool(name="wfc", bufs=8) as wfc_pool, \
             tc.tile_pool(name="wpj", bufs=3) as wpj_pool, \
             tc.tile_pool(name="lr", bufs=4) as lr_pool, \
             tc.tile_pool(name="ps", bufs=8, space="PSUM") as ps_pool, \
             tc.tile_pool(name="ot", bufs=4) as out_pool:
            aT = a_pool.tile([P, KB2, T], dt.bfloat16)
            xs_sb = xs_pool.tile([P, KB1, T], dt.bfloat16)

            # HAM pre-warm: one SHORT-window (~3.4us at the cold 1.2 GHz
            # clock, i.e. 6 MMs at ~630ns) of dummy matmuls on zeroed
            # tiles runs during the DMA lead-in, flipping the PE clock
            # gate to 8/8 before the first real matmul.  Zeroing rides
            # GpSimd, which boots ~2.5us before ScalarE clears its
            # ACT_TABLE_LOAD; more dummies would outlive the data wait
            # and push the real stream later (measured at 16 dummies).
            warm_w = lr_pool.tile([P, P], dt.bfloat16, tag="ww")
            warm_x = lr_pool.tile([P, 512], dt.bfloat16, tag="wx")
            nc.gpsimd.memset(warm_w, 0)
            nc.gpsimd.memset(warm_x, 0)
            warm_ps = ps_pool.tile([P, 512], dt.float32, tag="ps", name="wps")
            for _ in range(6):
                nc.tensor.matmul(warm_ps, warm_w, warm_x, start=True, stop=True)

            # First-data DMAs, finest-grained: the first matmul needs only
            # xs k-block 0 + the first quarter of wfc panel 0 (~0.26 MB),
            # not the whole input + panel (8 cores contend for HBM here).
            # wfc panel 0 quarters interleave with even xs k-blocks on the
            # sync ring; odd xs k-blocks ride the scalar ring.
            wfc0_sb = wfc_pool.tile([P, KB1, P], dt.bfloat16, tag="wfc",
                                    name="wfc0_sb")
            for k0 in range(0, KB1, 4):
                nc.sync.dma_start(wfc0_sb[:, k0:k0 + 4],
                                  wfcT_r[:, k0:k0 + 4, 0:P])
                for kb in range(k0, k0 + 4, 2):
                    nc.sync.dma_start(xs_sb[:, kb:kb + 1],
                                      xsT_r[:, kb:kb + 1])
            for kb in range(1, KB1, 2):
                nc.scalar.dma_start(xs_sb[:, kb:kb + 1], xsT_r[:, kb:kb + 1])

            # ---- phase 1: hT = wfc @ xs; aT = sq(lrelu(hT, 0.5)) ----
            # wfc panels (one h-block each) on the sync ring.
            for h in range(KB2):
                if h == 0:
                    wfc_sb = wfc0_sb
                else:
                    wfc_sb = wfc_pool.tile([P, KB1, P], dt.bfloat16,
                                           tag="wfc", name="wfc_sb")
                    nc.sync.dma_start(wfc_sb, wfcT_r[:, :, h * P:(h + 1) * P])

                pss = []
                for ti, tb in enumerate(tbs):
                    pss.append(ps_pool.tile([P, tb], dt.float32, tag="ps", name="ps"))
                for kb in range(KB1):
                    w = wfc_sb[:, kb, :]
                    for ti, tb in enumerate(tbs):
                        t0 = toff[ti]
                        nc.tensor.matmul(
                            pss[ti], w, xs_sb[:, kb, t0:t0 + tb],
                            start=(kb == 0), stop=(kb == KB1 - 1))
                for ti, tb in enumerate(tbs):
                    t0 = toff[ti]
                    if USE_LRELU:
                        lr = lr_pool.tile([P, tb], dt.float32, tag="lr")
                        nc.scalar.activation(
                            lr, pss[ti], mybir.ActivationFunctionType.Lrelu,
                            alpha=0.5)
                        nc.scalar.activation(
                            aT[:, h, t0:t0 + tb], lr,
                            mybir.ActivationFunctionType.Square)
                    else:
                        r = lr_pool.tile([P, tb], dt.float32, tag="r")
                        nc.scalar.activation(
                            r, pss[ti], mybir.ActivationFunctionType.Relu)
                        s = lr_pool.tile([P, tb], dt.float32, tag="s")
                        nc.vector.tensor_add(out=s, in0=pss[ti], in1=r)
                        nc.scalar.activation(
                            aT[:, h, t0:t0 + tb], s,
                            mybir.ActivationFunctionType.Square, scale=0.5)

            # ---- phase 2: yT = wpj @ aT ----
            # wpj panels (one d-block each) prefetch on the scalar ring.
            # Each panel DMA is gated behind phase-1 progress with a 1-elem
            # copy from aT (RAW on phase 1) into the panel buffer (WAW with
            # the DMA): without the gate the scheduler hoists all wpj DMAs
            # to t=0 and they crowd out the lead-in-critical xs/wfc loads
            # (measured +14us on the first matmul).
            for db in range(KB1):
                wpj_sb = wpj_pool.tile([P, KB2, P], dt.bfloat16, tag="wpj")
                if db < 3:
                    # Gate ONLY the first bufs panels (fresh buffers, no
                    # WAR): a 1-elem copy from aT h-block db (RAW on early
                    # phase 1) into the panel (WAW with its DMA) keeps the
                    # prefetch off the lead-in-critical xs/wfc window.
                    # Gating later panels would add a WAR-on-phase-2 wait
                    # to the strict-FIFO ACT queue ahead of phase-1
                    # activations -> PSUM starvation deadlock (measured:
                    # NRT_EXEC_UNIT_UNRECOVERABLE); those panels are
                    # already throttled by pool-buffer recycling.
                    nc.scalar.copy(wpj_sb[0:1, 0, 0:1],
                                   aT[0:1, 12 + 16 * db, 0:1])
                nc.scalar.dma_start(wpj_sb, wpjT_r[:, :, db * P:(db + 1) * P])

                pss = []
                for ti, tb in enumerate(tbs):
                    pss.append(ps_pool.tile([P, tb], dt.float32, tag="ps", name="ps"))
                for kb in range(KB2):
                    w = wpj_sb[:, kb, :]
                    for ti, tb in enumerate(tbs):
                        t0 = toff[ti]
                        nc.tensor.matmul(
                            pss[ti], w, aT[:, kb, t0:t0 + tb],
                            start=(kb == 0), stop=(kb == KB2 - 1))
                ot = out_pool.tile([P, T], dt.float32, tag="o")
                for ti, tb in enumerate(tbs):
                    t0 = toff[ti]
                    nc.scalar.copy(ot[:, t0:t0 + tb], pss[ti])
                nc.sync.dma_start(yT_r[:, db, :], ot)
    if LDW_DEDUP:
        _dedup_ldweights(nc)
    nc.compile()
    return nc


def get_nc(T, tbs):
    key = (T, tbs)
    if key not in _NC_CACHE:
        _NC_CACHE[key] = _build_nc(DIM, HID, T, tbs)
    return _NC_CACHE[key]


# --------------------------------------------------------------------------
# runner: build the sharded jit once per nc, reuse across calls
# --------------------------------------------------------------------------
def get_runner(nc, n_cores=NCORES):
    """Returns (fn, in_names, out_names, out_shapes).  fn takes
    [n_cores*dim0, ...] concatenated inputs + zero output buffers and
    returns concatenated outputs (mirrors bass2jax.run_bass_via_pjrt,
    but the jitted callable is cached so repeat calls don't recompile)."""
    key = id(nc)
    if key in _RUN_CACHE:
        return _RUN_CACHE[key]

    import jax
    import concourse.mybir as mybir
    from concourse.bass2jax import (
        _bass_exec_p, install_neuronx_cc_hook, partition_id_tensor)
    from jax.sharding import Mesh, PartitionSpec
    try:
        from jax.experimental.shard_map import shard_map
    except ImportError:
        from jax.shard_map import shard_map

    install_neuronx_cc_hook()

    part_name = (nc.partition_id_tensor.name
                 if nc.partition_id_tensor else None)
    in_names, out_names, out_avals = [], [], []
    for alloc in nc.m.functions[0].allocations:
        if not isinstance(alloc, mybir.MemoryLocationSet):
            continue
        name = alloc.memorylocations[0].name
        if alloc.kind == "ExternalInput":
            if name != part_name:
                in_names.append(name)
        elif alloc.kind == "ExternalOutput":
            out_names.append(name)
            out_avals.append(jax.core.ShapedArray(
                tuple(alloc.tensor_shape), mybir.dt.np(alloc.dtype)))
    n_params = len(in_names)
    n_outs = len(out_names)
    all_names = in_names + out_names
    if part_name is not None:
        all_names = all_names + [part_name]
    donate = tuple(range(n_params, n_params + n_outs))

    def _body(*args):
        operands = list(args)
        if part_name is not None:
            operands.append(partition_id_tensor())
        outs = _bass_exec_p.bind(
            *operands,
            out_avals=tuple(out_avals),
            in_names=tuple(all_names),
            out_names=tuple(out_names),
            lowering_input_output_aliases=(),
            sim_require_finite=True,
            sim_require_nnan=True,
            nc=nc,
        )
        return tuple(outs)

    devices = jax.devices()[:n_cores]
    mesh = Mesh(np.asarray(devices), ("core",))
    in_specs = (PartitionSpec("core"),) * (n_params + n_outs)
    out_specs = (PartitionSpec("core"),) * n_outs
    fn = jax.jit(
        shard_map(_body, mesh=mesh, in_specs=in_specs,
                  out_specs=out_specs, check_rep=False),
        donate_argnums=donate, keep_unused=True)
    out_shapes = [(tuple(a.shape), a.dtype) for a in out_avals]
    _RUN_CACHE[key] = (fn, in_names, out_names, out_shapes)
    return _RUN_CACHE[key]


def run_spmd(nc, in_maps, n_cores=NCORES):
    fn, in_names, out_names, out_shapes = get_runner(nc, n_cores)
    concat_in = [np.concatenate([m[n] for m in in_maps], axis=0)
                 for n in in_names]
    zeros = [np.zeros((n_cores * sh[0], *sh[1:]), dt)
             for sh, dt in out_shapes]
    outs = fn(*concat_in, *zeros)
    res = []
    for c in range(n_cores):
        res.append({
            name: np.asarray(outs[i]).reshape(n_cores, *out_shapes[i][0])[c]
            for i, name in enumerate(out_names)})
    return res


# --------------------------------------------------------------------------
# host dispatch
# --------------------------------------------------------------------------
def _route(x, w_router):
    """fp32 router matching reference: top = argmax(logits) (tie -> 0),
    s = top softmax prob = sigmoid(l_top - l_other)."""
    x_flat = np.asarray(x, dtype=np.float32).reshape(-1, x.shape[-1])
    L = x_flat @ np.asarray(w_router, dtype=np.float32).T
    top = (L[:, 1] > L[:, 0])
    dlt = np.abs(L[:, 1] - L[:, 0]).astype(np.float32)
    ptop = 1.0 / (1.0 + np.exp(-dlt))
    return x_flat, top, np.sqrt(ptop).astype(np.float32)


def _wT(w, e):
    bf16 = ml_dtypes.bfloat16
    return np.ascontiguousarray(np.asarray(w[e], np.float32).T).astype(bf16)


def _tbs_for(T):
    if T <= 512:
        return (T,)
    h = (T // 2 + 15) // 16 * 16
    return (h, T - h)


def prepare(x, w_router, w_fc, w_proj):
    """Host-side dispatch: returns (nc, in_maps, post) where
    post(res_list) assembles the full [B,S,D] output."""
    bsz, seq, d = x.shape
    N = bsz * seq
    assert d == DIM and N == NTOK
    bf16 = ml_dtypes.bfloat16

    x_flat, top, sq = _route(x, w_router)
    n1 = int(top.sum())
    n0 = N - n1
    cpe = NCORES // NEXP                       # cores per expert
    T = max(32, (max(n0, n1) + cpe - 1) // cpe)
    T = (T + 15) // 16 * 16
    if T_CAP < T <= T_MAX_SPARSE and max(n0, n1) - cpe * T_CAP <= SPILL_MAX:
        # cap device capacity at the balanced 512/core; the few overflow
        # tokens of the majority expert run on the host (exact fp32)
        T = T_CAP

    wfcT = [_wT(w_fc, e) for e in range(NEXP)]
    wpjT = [_wT(w_proj, e) for e in range(NEXP)]

    if T <= T_MAX_SPARSE:
        # ---- sparse: sort tokens by expert, 4 cores per expert ----
        perm0 = np.nonzero(~top)[0]
        perm1 = np.nonzero(top)[0]
        xs_scaled = x_flat * sq[:, None]
        cap = cpe * T
        keep0, keep1 = min(n0, cap), min(n1, cap)
        spills = [perm0[keep0:], perm1[keep1:]]
        xs_all = np.zeros((NCORES * T, DIM), dtype=np.float32)
        tok_of_slot = np.full(NCORES * T, -1, dtype=np.int64)
        xs_all[:keep0] = xs_scaled[perm0[:keep0]]
        tok_of_slot[:keep0] = perm0[:keep0]
        xs_all[cap:cap + keep1] = xs_scaled[perm1[:keep1]]
        tok_of_slot[cap:cap + keep1] = perm1[:keep1]

        in_maps = []
        for c in range(NCORES):
            e = 0 if c < cpe else 1
            xsT = np.ascontiguousarray(
                xs_all[c * T:(c + 1) * T].T).astype(bf16)
            in_maps.append({"xsT": xsT, "wfcT": wfcT[e], "wpjT": wpjT[e]})

        nc = get_nc(T, _tbs_for(T))

        def post(res):
            out_flat = np.zeros((N, DIM), dtype=np.float32)
            for c in range(NCORES):
                toks = tok_of_slot[c * T:(c + 1) * T]
                valid = toks >= 0
                if valid.any():
                    out_flat[toks[valid]] = res[c]["yT"].T[valid]
            for e, sp in enumerate(spills):
                if len(sp):
                    h = xs_scaled[sp] @ np.asarray(
                        w_fc[e], np.float32).reshape(HID, DIM).T
                    a = np.square(np.where(h >= 0, h, 0.5 * h))
                    out_flat[sp] = a @ np.asarray(
                        w_proj[e], np.float32).reshape(DIM, HID).T
            return out_flat.reshape(bsz, seq, d)

        return nc, in_maps, post

    # ---- dense fallback: token groups x experts ----
    sq0 = np.where(top, 0.0, sq).astype(np.float32)
    sq1 = np.where(top, sq, 0.0).astype(np.float32)
    in_maps = []
    for c in range(NCORES):
        g, e = c // NEXP, c % NEXP
        rows = slice(g * TPC_DENSE, (g + 1) * TPC_DENSE)
        xs = x_flat[rows] * (sq1 if e else sq0)[rows][:, None]
        xsT = np.ascontiguousarray(xs.T).astype(bf16)
        in_maps.append({"xsT": xsT, "wfcT": wfcT[e], "wpjT": wpjT[e]})

    nc = get_nc(TPC_DENSE, (512, 512))

    def post(res):
        out_flat = np.empty((N, DIM), dtype=np.float32)
        for g in range(NCORES // NEXP):
            yT0 = res[NEXP * g]["yT"]
            yT1 = res[NEXP * g + 1]["yT"]
            out_flat[g * TPC_DENSE:(g + 1) * TPC_DENSE] = (yT0 + yT1).T
        return out_flat.reshape(bsz, seq, d)

    return nc, in_maps, post


def kernel(x, w_router, w_fc, w_proj):
    nc, in_maps, post = prepare(x, w_router, w_fc, w_proj)
    res = run_spmd(nc, in_maps)
    return post(res)
